# revision 19
# baseline (speedup 1.0000x reference)
"""Single-head full-attention layer on 8 Trainium2 NeuronCores (fp8 DoubleRow).

reference:
    q = seq @ Wq; k = seq @ Wk; v = seq @ Wv          # [B,S,D], D=1024
    scores = q @ k.T / sqrt(D)                        # [B,S,S]
    out = seq + softmax(scores) @ v * mask            # [B,S,D]

Sharding: 8 cores = 4 batches x 2 sequence-halves, each core owning 1024
queries.  The K projection never runs on device: the host folds
M = Wq @ Wk^T (scores = seq @ M @ seq_full^T).  The full transposed
sequence (2MB fp8) is REPLICATED to every core as an input, in per-core
"local-first" key order [own 1024 keys, peer 1024 keys] (softmax+PV are
key-permutation invariant), so the scores phase never waits on a
collective and the SPMD program stays identical on all cores.  The only
on-device exchange is V: each core projects V for its own keys and
swaps halves with its pair-partner via two AllGathers that complete
long before the O phase reads the peer V (the peer halves are read with
dynamic-slot DMA slices, the slot index coming from a per-core host
input).

All inputs are pre-rearranged on the host into the SBUF-tile layout
[128 partitions, ksub-blocked free dim] so every input DMA is a dense
per-partition contiguous copy (max line size, minimal descriptors), all
on the sync queue (whose 16-engine fanout sustains full HBM bandwidth).
Blocks are ordered first-needed-first and the V phase iterates n-outer,
so V's first matmul group needs only the first 1MB (own seq columns
0..511 + Wv output-columns 0..511) and later blocks stream in under
compute.

All matmuls run in fp8(E4M3) with perf_mode=DoubleRow (two contraction
rows per PE cell): operands live in 3D SBUF tiles [128, ksub, free] and
each matmul consumes a [:, k:k+2, :] slice. Numerics:
  - host scales M and Wv by 32 before the fp8 cast; seq is cast raw
    (N(0,1) fits fp8).  Q' = seq @ M lands at 32x, exactly like the old
    q, so exp's scale is 2^-10 (1/sqrt(D) * 1/32).
  - exp is shifted by -3 so attn values stay below fp8e4's +-240 max
    (softmax is shift-invariant).
  - scores/colsum/out accumulate in fp32 PSUM; the last matmul emits
    the output in [q, d] layout so 1/colsum is a per-partition scalar
    and normalize + residual-add (bf16 residual) fuse into one
    scalar_tensor_tensor.  The output mask is folded into Wv on the
    host.  The colsum PSUM tile is [2, 512] (one bank) so the matmul
    pool can hold 7 banks.
"""

import numpy as np
import ml_dtypes

import concourse.bass as bass
import concourse.mybir as mybir
import concourse.tile as tile
from concourse import bacc, bass_utils

B, S, D = 4, 2048, 1024
N_CORES = 8
SH = S // 2          # queries / own keys per core
PD = 128             # partition dim
KD = D // PD         # 8 ksub chunks over d
KH = SH // PD        # 8 ksub chunks over own keys
KC = S // PD         # 16 ksub chunks over all keys
NT = 512             # matmul free-dim tile (one PSUM bank of fp32)
HT = SH // 2         # 512: half-block split of the own seq columns
F8 = mybir.dt.float8e4
F32 = mybir.dt.float32
BF16 = mybir.dt.bfloat16
W_SCALE = 32.0
EXP_SCALE = 1.0 / (32.0 * W_SCALE)     # 1/sqrt(D) / W_SCALE
EXP_SHIFT = -3.0
DR = mybir.MatmulPerfMode.DoubleRow
WARMUP = 46          # N=128 dummy matmuls: ~100% PE duty so HAM warms

_FP8 = ml_dtypes.float8_e4m3
_GROUPS = [[0, 1], [2, 3], [4, 5], [6, 7]]


def _build_kernel(tc):
    nc = tc.nc
    # host-packed inputs: [128, blocked free]; see _pack in _prep_in_maps
    seqT2 = nc.dram_tensor("seqT2", [PD, KD * S], F8, kind="ExternalInput").ap()
    m3w = nc.dram_tensor("m3w", [PD, KD * D], F8, kind="ExternalInput").ap()
    wvp = nc.dram_tensor("wvp", [PD, KD * D], F8, kind="ExternalInput").ap()
    shp = nc.dram_tensor("shp", [PD, KH * D], BF16, kind="ExternalInput").ap()
    peer_t = nc.dram_tensor("peer", [1, 1], mybir.dt.uint32, kind="ExternalInput")
    outT = nc.dram_tensor("outT", [SH, D], BF16, kind="ExternalOutput").ap()

    Exp = mybir.ActivationFunctionType.Exp

    with (
        tc.tile_pool(name="p_seq", bufs=1) as p_seq,
        tc.tile_pool(name="p_w", bufs=2) as p_w,
        tc.tile_pool(name="p_qt", bufs=1) as p_qt,
        tc.tile_pool(name="p_vo", bufs=1) as p_vo,
        tc.tile_pool(name="p_vp", bufs=1) as p_vp,
        tc.tile_pool(name="p_at", bufs=1) as p_at,
        tc.tile_pool(name="p_sh", bufs=1) as p_sh,
        tc.tile_pool(name="p_o", bufs=4) as p_o,
        tc.tile_pool(name="p_msc", bufs=1) as p_msc,
        tc.tile_pool(name="p_dram", bufs=1, space="DRAM") as p_dram,
        tc.tile_pool(name="p_mm", bufs=6, space="PSUM") as p_mm,
        tc.tile_pool(name="p_cs", bufs=1, space="PSUM") as p_cs,
    ):
        # ---- warm-up seed on the (otherwise idle) gpsimd queue: the vector
        # queue's preamble includes a ~1us iram load, which would delay the
        # PE's first warm-up matmul by ~2.5us
        wu_sb = p_msc.tile([PD, 2, PD], F8, tag="wu", name="wu_sb")
        nc.gpsimd.memset(wu_sb[:], 0.0)
        # constants seeded up-front on the idle gpsimd queue: on the vector
        # queue they would sit behind all 16 Q'-phase casts, stalling the
        # first colsum matmul (ones32) and the first exp (ebias) ~2.4us
        ones3 = p_msc.tile([PD, 2, 16], F8, tag="ones", name="ones3")
        nc.gpsimd.memset(ones3[:], float(W_SCALE))
        ebias = p_msc.tile([PD, 1], F32, tag="ebias", name="ebias")
        nc.gpsimd.memset(ebias[:], EXP_SHIFT)

        # ---- resident inputs, first-needed blocks first.  V(n=0, m<4)
        # needs only blocks A+B (1MB); C..D stream in under V compute.
        seq3 = p_seq.tile([PD, KD, S], F8, tag="seq", name="seq3")
        wv3 = p_w.tile([PD, KD, D], F8, tag="wv", name="wv3")
        nc.sync.dma_start(           # A: own seq cols 0..511
            seq3[:, :, 0:HT],
            seqT2[:, 0:KD * HT].rearrange("p (j s) -> p j s", j=KD))
        nc.sync.dma_start(           # B: Wv out-cols 0..511
            wv3[:, :, 0:NT],
            wvp[:, 0:KD * NT].rearrange("p (j d) -> p j d", j=KD))
        nc.sync.dma_start(           # C: own seq cols 512..1023
            seq3[:, :, HT:SH],
            seqT2[:, KD * HT:KD * SH].rearrange("p (j s) -> p j s", j=KD))
        nc.sync.dma_start(           # D: Wv out-cols 512..1023
            wv3[:, :, NT:D],
            wvp[:, KD * NT:KD * D].rearrange("p (j d) -> p j d", j=KD))
        m3 = p_w.tile([PD, KD, D], F8, tag="m3", name="m3")
        nc.sync.dma_start(m3[:], m3w.rearrange("p (j d) -> p j d", j=KD))
        sh3 = p_sh.tile([PD, KH, D], BF16, tag="sh", name="sh3")

        # V is exchanged as two half-size AllGathers so the first half
        # lands before the O phase touches the peer V.
        ib_v = p_dram.tile([SH, D], F8, tag="ibv", name="ib_v")
        ob_v1 = p_dram.tile([2, SH // 2, D], F8, tag="obv1", name="ob_v1")
        ob_v2 = p_dram.tile([2, SH // 2, D], F8, tag="obv2", name="ob_v2")

        # ---- HAM warm-up: N=128 streaming keeps the PE at ~100% duty
        # during the input-DMA wait so the clock gate is at 2.4GHz when
        # real work starts (N=16 tiles at the NX issue floor only reach
        # ~40% duty and never trip the activity monitor).
        ps_wu = p_mm.tile([PD, PD], F32, tag="mm", name="ps_wu")
        for i in range(WARMUP):
            nc.tensor.matmul(
                ps_wu[:], wu_sb[:], wu_sb[:],
                start=(i == 0), stop=(i == WARMUP - 1), perf_mode=DR,
            )

        # ---- V_own = seq_own @ (Wv * mask), bounce out, AllGather ----------
        # n-outer so the first 8 matmul groups touch only Wv's low output
        # half (block B); the bounce of row-block m needs both halves so it
        # lives in the n==1 pass.
        vo3 = p_vo.tile([PD, KH, D], F8, tag="vo", name="vo3")
        for n in range(D // NT):
            for m in range(KH):
                ps = p_mm.tile([PD, NT], F32, tag="mm", name=f"ps_v{m}_{n}")
                for k in range(0, KD, 2):
                    nc.tensor.matmul(
                        ps[:],
                        seq3[:, k:k + 2, m * PD:(m + 1) * PD],
                        wv3[:, k:k + 2, n * NT:(n + 1) * NT],
                        start=(k == 0),
                        stop=(k == KD - 2),
                        perf_mode=DR,
                    )
                nc.vector.tensor_copy(vo3[:, m, n * NT:(n + 1) * NT], ps[:])
                if n == 1:
                    nc.sync.dma_start(ib_v[m * PD:(m + 1) * PD, :], vo3[:, m, :])
                    if m == KH // 2 - 1:
                        nc.gpsimd.collective_compute(
                            "AllGather", mybir.AluOpType.bypass,
                            replica_groups=_GROUPS,
                            ins=[ib_v[0:SH // 2, :]], outs=[ob_v1.opt()],
                        )
        nc.gpsimd.collective_compute(
            "AllGather", mybir.AluOpType.bypass, replica_groups=_GROUPS,
            ins=[ib_v[SH // 2:SH, :]], outs=[ob_v2.opt()],
        )

        # late inputs, issued after the V phase so they don't steal DMA
        # bandwidth from the V-critical first 3MB.  The peer-key seq
        # columns are first needed by scores chunk m=KD (~25us later);
        # the residual by the O phase (~55us later).
        nc.sync.dma_start(
            seq3[:, :, SH:S],
            seqT2[:, KD * SH:KD * S].rearrange("p (j s) -> p j s", j=KD))
        nc.sync.dma_start(sh3[:], shp.rearrange("p (j d) -> p j d", j=KD))

        # ---- Q'^T = (seq_own @ M).T with M = Wq Wk^T folded on the host ----
        qt3 = p_qt.tile([PD, KD, SH], F8, tag="qt", name="qt3")
        for m in range(KD):
            for n in range(SH // NT):
                ps = p_mm.tile([PD, NT], F32, tag="mm", name=f"ps_q{m}_{n}")
                for k in range(0, KD, 2):
                    nc.tensor.matmul(
                        ps[:],
                        m3[:, k:k + 2, m * PD:(m + 1) * PD],
                        seq3[:, k:k + 2, n * NT:(n + 1) * NT],
                        start=(k == 0),
                        stop=(k == KD - 2),
                        perf_mode=DR,
                    )
                nc.vector.tensor_copy(qt3[:, m, n * NT:(n + 1) * NT], ps[:])

        # ---- peer V halves, via dynamic-slot DMA slices.  peer_slot (0|1)
        # is a per-core host input; the sync engine blocks here until each
        # AllGather lands -- long before the O phase needs the data.
        preg = nc.sync.alloc_register("peer_slot")
        nc.sync.reg_load(preg, peer_t[0:1, 0:1])
        pslot = nc.sync.snap(preg, donate=True, min_val=0, max_val=1)
        v_other = p_vp.tile([PD, KH, D], F8, tag="vp", name="v_other")
        nc.sync.dma_start(
            v_other[:, 0:KH // 2, :],
            ob_v1[bass.ds(pslot, 1), :, :].rearrange(
                "o (j p) d -> (o p) j d", p=PD),
        )
        nc.sync.dma_start(
            v_other[:, KH // 2:KH, :],
            ob_v2[bass.ds(pslot, 1), :, :].rearrange(
                "o (j p) d -> (o p) j d", p=PD),
        )

        # ---- scoresT -> exp(shifted) -> colsum, local-first key order ------
        # 32.0 (exact in fp8, memset up-front) folds V's W_SCALE into the
        # colsum so the reciprocal needs no extra rescale
        cs_ps = p_cs.tile([1, SH], F32, tag="cs", name="cs")
        at3 = p_at.tile([PD, KC, SH], F8, tag="at", name="at3")

        def colsum_mm(m):
            for n in range(SH // NT):
                nc.tensor.matmul(
                    cs_ps[:, n * NT:(n + 1) * NT],
                    ones3[:, 0:2, 0:1],
                    at3[:, m:m + 2, n * NT:(n + 1) * NT],
                    start=(m == 0),
                    stop=(m == KC - 2),
                    perf_mode=DR,
                )

        for m in range(KC):
            for n in range(SH // NT):
                ps = p_mm.tile([PD, NT], F32, tag="mm", name=f"ps_s{m}_{n}")
                for k in range(0, KD, 2):
                    nc.tensor.matmul(
                        ps[:],
                        seq3[:, k:k + 2, m * PD:(m + 1) * PD],
                        qt3[:, k:k + 2, n * NT:(n + 1) * NT],
                        start=(k == 0),
                        stop=(k == KD - 2),
                        perf_mode=DR,
                    )
                nc.scalar.activation(
                    at3[:, m, n * NT:(n + 1) * NT], ps[:], Exp,
                    bias=ebias[:], scale=EXP_SCALE,
                )
            # colsum pairs ksubs (m, m+1); emit one pair late so the PE
            # never waits on ACT's exp.  The final pair (needing the last
            # exps) is deferred into the O phase so the PE chews O work
            # instead of idling on the ACT tail.
            if m >= 3 and m % 2 == 1:
                colsum_mm(m - 3)

        # ---- 1/(32*colsum) transposed to per-partition [128, 8] -------------
        # reciprocal on [1, SH] runs on one DVE lane (6.5us); transpose the
        # colsum to [128, 8] via a DRAM bounce first so it takes ~0.2us.
        cs_sb = p_msc.tile([1, SH], F32, tag="cs_sb", name="cs_sb")
        nc.vector.tensor_copy(cs_sb[:], cs_ps[:])
        cs_d = p_dram.tile([1, SH], F32, tag="csd", name="cs_d")
        nc.gpsimd.dma_start(cs_d[:], cs_sb[:])
        csT = p_msc.tile([PD, KH], F32, tag="csT", name="csT")
        nc.gpsimd.dma_start(csT[:, :], cs_d.rearrange("o (m p) -> (o p) m", p=PD))
        recipT = p_msc.tile([PD, KH], F32, tag="recipT", name="recipT")
        nc.vector.reciprocal(recipT[:, :], csT[:, :])

        # ---- O = AT.T @ V in [q, d] layout; fused normalize + residual ------
        # out[q, d] = (sum_key at[key, q] * v[key, d]) * recip[q] + seq[q, d]
        # key chunk pairs 0..7 stream from vo3 (own V), 8..15 from v_other.
        for m in range(KH):
            o_t = p_o.tile([PD, D], BF16, tag="o", name=f"o{m}")
            for n in range(D // NT):
                ps = p_mm.tile([PD, NT], F32, tag="mm", name=f"ps_o{m}_{n}")
                for k in range(0, KC, 2):
                    v3t = vo3 if k < KH else v_other
                    kk = k if k < KH else k - KH
                    nc.tensor.matmul(
                        ps[:],
                        at3[:, k:k + 2, m * PD:(m + 1) * PD],
                        v3t[:, kk:kk + 2, n * NT:(n + 1) * NT],
                        start=(k == 0), stop=(k == KC - 2), perf_mode=DR,
                    )
                if m == 0 and n == 0:
                    colsum_mm(KC - 2)
                nc.vector.scalar_tensor_tensor(
                    o_t[:, n * NT:(n + 1) * NT],
                    ps[:],
                    recipT[:, m:m + 1],
                    sh3[:, m, n * NT:(n + 1) * NT],
                    op0=mybir.AluOpType.mult,
                    op1=mybir.AluOpType.add,
                )
                if m == KH - 1:
                    # last tile: store each half as soon as its stt lands so
                    # the final store is 256KB, not 512KB, off the tail
                    nc.sync.dma_start(
                        outT[m * PD:(m + 1) * PD, n * NT:(n + 1) * NT],
                        o_t[:, n * NT:(n + 1) * NT])
            if m < KH - 1:
                nc.sync.dma_start(outT[m * PD:(m + 1) * PD, :], o_t[:])


_NC_CACHE = None


def _get_nc():
    global _NC_CACHE
    if _NC_CACHE is None:
        nc = bacc.Bacc(
            "TRN2", target_bir_lowering=False, debug=False, num_devices=N_CORES
        )
        with tile.TileContext(nc) as tc:
            _build_kernel(tc)
        nc.compile()
        _NC_CACHE = nc
    return _NC_CACHE


def _pack(a, blocks):
    """Pack [D_rows, cols] into [128, sum(j_rows * width)] tile layout,
    block-by-block: each (c0, c1) column block is laid out as
    [128, nrows/128, width] with row r = j*128 + p."""
    out = []
    for c0, c1 in blocks:
        blk = a[:, c0:c1]
        j = blk.shape[0] // PD
        out.append(blk.reshape(j, PD, c1 - c0).transpose(1, 0, 2).reshape(PD, -1))
    return np.ascontiguousarray(np.concatenate(out, axis=1))


def _prep_in_maps(seq, Wq, Wk, Wv, mask):
    seq = np.asarray(seq, dtype=np.float32)
    M = np.asarray(Wq, dtype=np.float32) @ np.asarray(Wk, dtype=np.float32).T
    m_f8 = (M * W_SCALE).astype(_FP8)
    wvm_f8 = (np.asarray(Wv, dtype=np.float32)
              * np.asarray(mask, dtype=np.float32)[None, :] * W_SCALE).astype(_FP8)
    m3p = _pack(m_f8, [(0, D)])
    wvp = _pack(wvm_f8, [(0, NT), (NT, D)])
    in_maps = []
    for c in range(N_CORES):
        b, h = divmod(c, 2)
        # full transposed seq in local-first key order: [own half, peer half]
        sT = seq[b].T.astype(_FP8)  # [D, S]
        own = sT[:, h * SH:(h + 1) * SH]
        peer = sT[:, (1 - h) * SH:(2 - h) * SH]
        seqT2 = np.concatenate(
            [_pack(own, [(0, HT), (HT, SH)]), _pack(peer, [(0, SH)])], axis=1)
        shp = _pack(np.ascontiguousarray(
            seq[b, h * SH:(h + 1) * SH, :]).astype(ml_dtypes.bfloat16), [(0, D)])
        in_maps.append({
            "seqT2": np.ascontiguousarray(seqT2),
            "m3w": m3p,
            "wvp": wvp,
            "shp": shp,
            "peer": np.array([[1 - h]], dtype=np.uint32),
        })
    return in_maps


def _run(seq, Wq, Wk, Wv, mask, trace=False, **run_kwargs):
    nc = _get_nc()
    in_maps = _prep_in_maps(seq, Wq, Wk, Wv, mask)
    res = bass_utils.run_bass_kernel_spmd(
        nc, in_maps, core_ids=list(range(N_CORES)), trace=trace, **run_kwargs
    )
    out = np.empty((B, S, D), dtype=np.float32)
    for c in range(N_CORES):
        b, h = divmod(c, 2)
        out[b, h * SH:(h + 1) * SH, :] = np.asarray(
            res.results[c]["outT"]).astype(np.float32)
    return out, res


def kernel(seq, Wq, Wk, Wv, mask):
    out, _ = _run(seq, Wq, Wk, Wv, mask)
    return out


# revision 21
# speedup vs baseline: 1.0342x; 1.0342x over previous
"""Single-head full-attention layer on 8 Trainium2 NeuronCores (fp8 DoubleRow).

reference:
    q = seq @ Wq; k = seq @ Wk; v = seq @ Wv          # [B,S,D], D=1024
    scores = q @ k.T / sqrt(D)                        # [B,S,S]
    out = seq + softmax(scores) @ v * mask            # [B,S,D]

Sharding: 8 cores = 4 batches x 2 sequence-halves, each core owning 1024
queries.  The K projection never runs on device: the host folds
M = Wq @ Wk^T (scores = seq @ M @ seq_full^T).  The full transposed
sequence (2MB fp8) is REPLICATED to every core as an input, in per-core
"local-first" key order [own 1024 keys, peer 1024 keys] (softmax+PV are
key-permutation invariant), so the scores phase never waits on a
collective and the SPMD program stays identical on all cores.  The only
on-device exchange is V: each core projects V for its own keys and
swaps halves with its pair-partner via two AllGathers that complete
long before the O phase reads the peer V (the peer halves are read with
dynamic-slot DMA slices, the slot index coming from a per-core host
input).

All inputs are pre-rearranged on the host into the SBUF-tile layout
[128 partitions, ksub-blocked free dim] so every input DMA is a dense
per-partition contiguous copy (max line size, minimal descriptors), all
on the sync queue (whose 16-engine fanout sustains full HBM bandwidth).
Blocks are ordered first-needed-first and the V phase iterates n-outer,
so V's first matmul group needs only the first 1MB (own seq columns
0..511 + Wv output-columns 0..511) and later blocks stream in under
compute.

All matmuls run in fp8(E4M3) with perf_mode=DoubleRow (two contraction
rows per PE cell): operands live in 3D SBUF tiles [128, ksub, free] and
each matmul consumes a [:, k:k+2, :] slice. Numerics:
  - host scales M and Wv by 32 before the fp8 cast; seq is cast raw
    (N(0,1) fits fp8).  Q' = seq @ M lands at 32x, exactly like the old
    q, so exp's scale is 2^-10 (1/sqrt(D) * 1/32).
  - exp is shifted by -3 so attn values stay below fp8e4's +-240 max
    (softmax is shift-invariant).
  - scores/colsum/out accumulate in fp32 PSUM; the last matmul emits
    the output in [q, d] layout so 1/colsum is a per-partition scalar
    and normalize + residual-add (bf16 residual) fuse into one
    scalar_tensor_tensor.  The output mask is folded into Wv on the
    host.  The colsum PSUM tile is [2, 512] (one bank) so the matmul
    pool can hold 7 banks.
"""

import numpy as np
import ml_dtypes

import concourse.bass as bass
import concourse.mybir as mybir
import concourse.tile as tile
from concourse import bacc, bass_utils

B, S, D = 4, 2048, 1024
N_CORES = 8
SH = S // 2          # queries / own keys per core
PD = 128             # partition dim
KD = D // PD         # 8 ksub chunks over d
KH = SH // PD        # 8 ksub chunks over own keys
KC = S // PD         # 16 ksub chunks over all keys
NT = 512             # matmul free-dim tile (one PSUM bank of fp32)
HT = SH // 2         # 512: half-block split of the own seq columns
F8 = mybir.dt.float8e4
F32 = mybir.dt.float32
BF16 = mybir.dt.bfloat16
W_SCALE = 32.0
EXP_SCALE = 1.0 / (32.0 * W_SCALE)     # 1/sqrt(D) / W_SCALE
EXP_SHIFT = -3.0
DR = mybir.MatmulPerfMode.DoubleRow
WARMUP = 46          # N=128 dummy matmuls: ~100% PE duty so HAM warms

_FP8 = ml_dtypes.float8_e4m3
_GROUPS = [[0, 1], [2, 3], [4, 5], [6, 7]]


def _build_kernel(tc):
    nc = tc.nc
    # host-packed inputs: [128, blocked free]; see _pack in _prep_in_maps
    seqT2 = nc.dram_tensor("seqT2", [PD, KD * S], F8, kind="ExternalInput").ap()
    m3w = nc.dram_tensor("m3w", [PD, KD * D], F8, kind="ExternalInput").ap()
    wvp = nc.dram_tensor("wvp", [PD, KD * D], F8, kind="ExternalInput").ap()
    shp = nc.dram_tensor("shp", [PD, KH * D], BF16, kind="ExternalInput").ap()
    peer_t = nc.dram_tensor("peer", [1, 1], mybir.dt.uint32, kind="ExternalInput")
    outT = nc.dram_tensor("outT", [SH, D], BF16, kind="ExternalOutput").ap()

    Exp = mybir.ActivationFunctionType.Exp

    with (
        tc.tile_pool(name="p_seq", bufs=1) as p_seq,
        tc.tile_pool(name="p_w", bufs=2) as p_w,
        tc.tile_pool(name="p_qt", bufs=1) as p_qt,
        tc.tile_pool(name="p_vo", bufs=1) as p_vo,
        tc.tile_pool(name="p_vp", bufs=1) as p_vp,
        tc.tile_pool(name="p_at", bufs=1) as p_at,
        tc.tile_pool(name="p_sh", bufs=1) as p_sh,
        tc.tile_pool(name="p_o", bufs=4) as p_o,
        tc.tile_pool(name="p_msc", bufs=1) as p_msc,
        tc.tile_pool(name="p_dram", bufs=1, space="DRAM") as p_dram,
        tc.tile_pool(name="p_mm", bufs=6, space="PSUM") as p_mm,
        tc.tile_pool(name="p_cs", bufs=1, space="PSUM") as p_cs,
    ):
        # ---- warm-up seed first on the vector queue so the PE can start
        # right after its preamble
        wu_sb = p_msc.tile([PD, 2, PD], F8, tag="wu", name="wu_sb")
        nc.vector.memset(wu_sb[:], 0.0)

        # ---- resident inputs, first-needed blocks first.  V(n=0, m<4)
        # needs only blocks A+B (1MB); C..D stream in under V compute.
        seq3 = p_seq.tile([PD, KD, S], F8, tag="seq", name="seq3")
        wv3 = p_w.tile([PD, KD, D], F8, tag="wv", name="wv3")
        nc.sync.dma_start(           # A: own seq cols 0..511
            seq3[:, :, 0:HT],
            seqT2[:, 0:KD * HT].rearrange("p (j s) -> p j s", j=KD))
        nc.sync.dma_start(           # B: Wv out-cols 0..511
            wv3[:, :, 0:NT],
            wvp[:, 0:KD * NT].rearrange("p (j d) -> p j d", j=KD))
        nc.sync.dma_start(           # C: own seq cols 512..1023
            seq3[:, :, HT:SH],
            seqT2[:, KD * HT:KD * SH].rearrange("p (j s) -> p j s", j=KD))
        nc.sync.dma_start(           # D: Wv out-cols 512..1023
            wv3[:, :, NT:D],
            wvp[:, KD * NT:KD * D].rearrange("p (j d) -> p j d", j=KD))
        m3 = p_w.tile([PD, KD, D], F8, tag="m3", name="m3")
        nc.sync.dma_start(m3[:], m3w.rearrange("p (j d) -> p j d", j=KD))
        sh3 = p_sh.tile([PD, KH, D], BF16, tag="sh", name="sh3")

        # V is exchanged as two half-size AllGathers so the first half
        # lands before the O phase touches the peer V.
        ib_v = p_dram.tile([SH, D], F8, tag="ibv", name="ib_v")
        ob_v1 = p_dram.tile([2, SH // 2, D], F8, tag="obv1", name="ob_v1")
        ob_v2 = p_dram.tile([2, SH // 2, D], F8, tag="obv2", name="ob_v2")

        # ---- HAM warm-up: N=128 streaming keeps the PE at ~100% duty
        # during the input-DMA wait so the clock gate is at 2.4GHz when
        # real work starts (N=16 tiles at the NX issue floor only reach
        # ~40% duty and never trip the activity monitor).
        ps_wu = p_mm.tile([PD, PD], F32, tag="mm", name="ps_wu")
        for i in range(WARMUP):
            nc.tensor.matmul(
                ps_wu[:], wu_sb[:], wu_sb[:],
                start=(i == 0), stop=(i == WARMUP - 1), perf_mode=DR,
            )

        # ---- V_own = seq_own @ (Wv * mask), bounce out, AllGather ----------
        # n-outer so the first 8 matmul groups touch only Wv's low output
        # half (block B); the bounce of row-block m needs both halves so it
        # lives in the n==1 pass.
        vo3 = p_vo.tile([PD, KH, D], F8, tag="vo", name="vo3")
        for n in range(D // NT):
            for m in range(KH):
                ps = p_mm.tile([PD, NT], F32, tag="mm", name=f"ps_v{m}_{n}")
                for k in range(0, KD, 2):
                    nc.tensor.matmul(
                        ps[:],
                        seq3[:, k:k + 2, m * PD:(m + 1) * PD],
                        wv3[:, k:k + 2, n * NT:(n + 1) * NT],
                        start=(k == 0),
                        stop=(k == KD - 2),
                        perf_mode=DR,
                    )
                nc.vector.tensor_copy(vo3[:, m, n * NT:(n + 1) * NT], ps[:])
                if n == 1:
                    nc.sync.dma_start(ib_v[m * PD:(m + 1) * PD, :], vo3[:, m, :])
                    if m == KH // 2 - 1:
                        nc.gpsimd.collective_compute(
                            "AllGather", mybir.AluOpType.bypass,
                            replica_groups=_GROUPS,
                            ins=[ib_v[0:SH // 2, :]], outs=[ob_v1.opt()],
                        )
        nc.gpsimd.collective_compute(
            "AllGather", mybir.AluOpType.bypass, replica_groups=_GROUPS,
            ins=[ib_v[SH // 2:SH, :]], outs=[ob_v2.opt()],
        )

        # late inputs, issued after the V phase so they don't steal DMA
        # bandwidth from the V-critical first 3MB.  The peer-key seq
        # columns are first needed by scores chunk m=KD (~25us later);
        # the residual by the O phase (~55us later).
        nc.sync.dma_start(
            seq3[:, :, SH:S],
            seqT2[:, KD * SH:KD * S].rearrange("p (j s) -> p j s", j=KD))
        nc.sync.dma_start(sh3[:], shp.rearrange("p (j d) -> p j d", j=KD))

        # ---- Q'^T = (seq_own @ M).T with M = Wq Wk^T folded on the host ----
        qt3 = p_qt.tile([PD, KD, SH], F8, tag="qt", name="qt3")
        for m in range(KD):
            for n in range(SH // NT):
                ps = p_mm.tile([PD, NT], F32, tag="mm", name=f"ps_q{m}_{n}")
                for k in range(0, KD, 2):
                    nc.tensor.matmul(
                        ps[:],
                        m3[:, k:k + 2, m * PD:(m + 1) * PD],
                        seq3[:, k:k + 2, n * NT:(n + 1) * NT],
                        start=(k == 0),
                        stop=(k == KD - 2),
                        perf_mode=DR,
                    )
                nc.vector.tensor_copy(qt3[:, m, n * NT:(n + 1) * NT], ps[:])

        # ---- peer V halves, via dynamic-slot DMA slices.  peer_slot (0|1)
        # is a per-core host input; the sync engine blocks here until each
        # AllGather lands -- long before the O phase needs the data.
        preg = nc.sync.alloc_register("peer_slot")
        nc.sync.reg_load(preg, peer_t[0:1, 0:1])
        pslot = nc.sync.snap(preg, donate=True, min_val=0, max_val=1)
        v_other = p_vp.tile([PD, KH, D], F8, tag="vp", name="v_other")
        nc.sync.dma_start(
            v_other[:, 0:KH // 2, :],
            ob_v1[bass.ds(pslot, 1), :, :].rearrange(
                "o (j p) d -> (o p) j d", p=PD),
        )
        nc.sync.dma_start(
            v_other[:, KH // 2:KH, :],
            ob_v2[bass.ds(pslot, 1), :, :].rearrange(
                "o (j p) d -> (o p) j d", p=PD),
        )

        # ---- scoresT -> exp(shifted) -> colsum, local-first key order ------
        # 32.0 (exact in fp8) folds V's W_SCALE into the colsum so the
        # reciprocal needs no extra rescale
        ones3 = p_msc.tile([PD, 2, 16], F8, tag="ones", name="ones3")
        nc.vector.memset(ones3[:], float(W_SCALE))
        ebias = p_msc.tile([PD, 1], F32, tag="ebias", name="ebias")
        nc.vector.memset(ebias[:], EXP_SHIFT)
        cs_ps = p_cs.tile([1, SH], F32, tag="cs", name="cs")
        at3 = p_at.tile([PD, KC, SH], F8, tag="at", name="at3")

        def colsum_mm(m):
            for n in range(SH // NT):
                nc.tensor.matmul(
                    cs_ps[:, n * NT:(n + 1) * NT],
                    ones3[:, 0:2, 0:1],
                    at3[:, m:m + 2, n * NT:(n + 1) * NT],
                    start=(m == 0),
                    stop=(m == KC - 2),
                    perf_mode=DR,
                )

        for m in range(KC):
            for n in range(SH // NT):
                ps = p_mm.tile([PD, NT], F32, tag="mm", name=f"ps_s{m}_{n}")
                for k in range(0, KD, 2):
                    nc.tensor.matmul(
                        ps[:],
                        seq3[:, k:k + 2, m * PD:(m + 1) * PD],
                        qt3[:, k:k + 2, n * NT:(n + 1) * NT],
                        start=(k == 0),
                        stop=(k == KD - 2),
                        perf_mode=DR,
                    )
                nc.scalar.activation(
                    at3[:, m, n * NT:(n + 1) * NT], ps[:], Exp,
                    bias=ebias[:], scale=EXP_SCALE,
                )
            # colsum pairs ksubs (m, m+1); emit one pair late so the PE
            # never waits on ACT's exp.  The final pair (needing the last
            # exps) is deferred into the O phase so the PE chews O work
            # instead of idling on the ACT tail.
            if m >= 3 and m % 2 == 1:
                colsum_mm(m - 3)

        # ---- 1/(32*colsum) transposed to per-partition [128, 8] -------------
        # reciprocal on [1, SH] runs on one DVE lane (6.5us); transpose the
        # colsum to [128, 8] via a DRAM bounce first so it takes ~0.2us.
        cs_sb = p_msc.tile([1, SH], F32, tag="cs_sb", name="cs_sb")
        nc.vector.tensor_copy(cs_sb[:], cs_ps[:])
        cs_d = p_dram.tile([1, SH], F32, tag="csd", name="cs_d")
        nc.gpsimd.dma_start(cs_d[:], cs_sb[:])
        csT = p_msc.tile([PD, KH], F32, tag="csT", name="csT")
        nc.gpsimd.dma_start(csT[:, :], cs_d.rearrange("o (m p) -> (o p) m", p=PD))
        recipT = p_msc.tile([PD, KH], F32, tag="recipT", name="recipT")
        nc.vector.reciprocal(recipT[:, :], csT[:, :])

        # ---- O = AT.T @ V in [q, d] layout; fused normalize + residual ------
        # out[q, d] = (sum_key at[key, q] * v[key, d]) * recip[q] + seq[q, d]
        # key chunk pairs 0..7 stream from vo3 (own V), 8..15 from v_other.
        for m in range(KH):
            o_t = p_o.tile([PD, D], BF16, tag="o", name=f"o{m}")
            for n in range(D // NT):
                ps = p_mm.tile([PD, NT], F32, tag="mm", name=f"ps_o{m}_{n}")
                for k in range(0, KC, 2):
                    v3t = vo3 if k < KH else v_other
                    kk = k if k < KH else k - KH
                    nc.tensor.matmul(
                        ps[:],
                        at3[:, k:k + 2, m * PD:(m + 1) * PD],
                        v3t[:, kk:kk + 2, n * NT:(n + 1) * NT],
                        start=(k == 0), stop=(k == KC - 2), perf_mode=DR,
                    )
                if m == 0 and n == 0:
                    colsum_mm(KC - 2)
                nc.vector.scalar_tensor_tensor(
                    o_t[:, n * NT:(n + 1) * NT],
                    ps[:],
                    recipT[:, m:m + 1],
                    sh3[:, m, n * NT:(n + 1) * NT],
                    op0=mybir.AluOpType.mult,
                    op1=mybir.AluOpType.add,
                )
                if m == KH - 1:
                    # last tile: store each half as soon as its stt lands so
                    # the final store is 256KB, not 512KB, off the tail
                    nc.sync.dma_start(
                        outT[m * PD:(m + 1) * PD, n * NT:(n + 1) * NT],
                        o_t[:, n * NT:(n + 1) * NT])
            if m < KH - 1:
                nc.sync.dma_start(outT[m * PD:(m + 1) * PD, :], o_t[:])


_NC_CACHE = None


def _get_nc():
    global _NC_CACHE
    if _NC_CACHE is None:
        nc = bacc.Bacc(
            "TRN2", target_bir_lowering=False, debug=False, num_devices=N_CORES
        )
        with tile.TileContext(nc) as tc:
            _build_kernel(tc)
        nc.compile()
        _NC_CACHE = nc
    return _NC_CACHE


def _pack(a, blocks):
    """Pack [D_rows, cols] into [128, sum(j_rows * width)] tile layout,
    block-by-block: each (c0, c1) column block is laid out as
    [128, nrows/128, width] with row r = j*128 + p."""
    out = []
    for c0, c1 in blocks:
        blk = a[:, c0:c1]
        j = blk.shape[0] // PD
        out.append(blk.reshape(j, PD, c1 - c0).transpose(1, 0, 2).reshape(PD, -1))
    return np.ascontiguousarray(np.concatenate(out, axis=1))


def _prep_in_maps(seq, Wq, Wk, Wv, mask):
    seq = np.asarray(seq, dtype=np.float32)
    M = np.asarray(Wq, dtype=np.float32) @ np.asarray(Wk, dtype=np.float32).T
    m_f8 = (M * W_SCALE).astype(_FP8)
    wvm_f8 = (np.asarray(Wv, dtype=np.float32)
              * np.asarray(mask, dtype=np.float32)[None, :] * W_SCALE).astype(_FP8)
    m3p = _pack(m_f8, [(0, D)])
    wvp = _pack(wvm_f8, [(0, NT), (NT, D)])
    in_maps = []
    for c in range(N_CORES):
        b, h = divmod(c, 2)
        # full transposed seq in local-first key order: [own half, peer half]
        sT = seq[b].T.astype(_FP8)  # [D, S]
        own = sT[:, h * SH:(h + 1) * SH]
        peer = sT[:, (1 - h) * SH:(2 - h) * SH]
        seqT2 = np.concatenate(
            [_pack(own, [(0, HT), (HT, SH)]), _pack(peer, [(0, SH)])], axis=1)
        shp = _pack(np.ascontiguousarray(
            seq[b, h * SH:(h + 1) * SH, :]).astype(ml_dtypes.bfloat16), [(0, D)])
        in_maps.append({
            "seqT2": np.ascontiguousarray(seqT2),
            "m3w": m3p,
            "wvp": wvp,
            "shp": shp,
            "peer": np.array([[1 - h]], dtype=np.uint32),
        })
    return in_maps


def _run(seq, Wq, Wk, Wv, mask, trace=False, **run_kwargs):
    nc = _get_nc()
    in_maps = _prep_in_maps(seq, Wq, Wk, Wv, mask)
    res = bass_utils.run_bass_kernel_spmd(
        nc, in_maps, core_ids=list(range(N_CORES)), trace=trace, **run_kwargs
    )
    out = np.empty((B, S, D), dtype=np.float32)
    for c in range(N_CORES):
        b, h = divmod(c, 2)
        out[b, h * SH:(h + 1) * SH, :] = np.asarray(
            res.results[c]["outT"]).astype(np.float32)
    return out, res


def kernel(seq, Wq, Wk, Wv, mask):
    out, _ = _run(seq, Wq, Wk, Wv, mask)
    return out
